# revision 5
# baseline (speedup 1.0000x reference)
"""Trainium2 Bass kernel for nn_CFDFVnewGCN (6-layer FVnewConv GNN).

Strategy: shard destination nodes (and their incoming edges) across 8 cores.
Nodes are permuted/degree-balanced into 49 windows of <=128 nodes per core.
Per 128-edge tile: f32r scaling matmul (edge_attr stationary, bias folded as
7th K-row), ACT relu, DVE message multiply (plane-major D layout, na cols
last), one-hot scatter matmul accumulating aggr[window, D] in PSUM.
Per window: PE transposes of aggr + fused output matmul (bias via const ones
row), tanh(+relu), DMA out. x is replicated via 4 chunked AllGathers per
layer (overlapping compute). Gather uses dma_gather with two offset views of
the x buffer (rows 0:32768 and 17232:50000) so indices fit int16.
"""
import sys
import numpy as np

for _p in ('/opt/trn_rl_repo', '/root/.axon_site/_ro/trn_rl_repo'):
    if _p not in sys.path:
        sys.path.insert(0, _p)

import concourse.bacc as bacc
import concourse.mybir as mybir
import concourse.tile as tile
from concourse.bass_utils import run_bass_kernel_spmd

F32 = mybir.dt.float32
F32R = mybir.dt.float32r
I16 = mybir.dt.int16
I32 = mybir.dt.int32
COPY = mybir.ActivationFunctionType.Copy
RELU = mybir.ActivationFunctionType.Relu
TANH = mybir.ActivationFunctionType.Tanh
MULT = mybir.AluOpType.mult
ISEQ = mybir.AluOpType.is_equal

NCORES = 8


class Cfg:
    def __init__(self, n_nodes=50000, n_edges=200000, hid=512, hs=3, ea=6,
                 out=3, n_ag_chunks=4):
        self.N = n_nodes
        self.E = n_edges
        self.HID = hid
        self.HS = hs
        self.EA = ea
        self.OUT = out
        self.NPC = self.N // NCORES              # nodes per core
        self.NWIN = (self.NPC + 127) // 128      # windows per core
        self.WSIZES = [128] * (self.NWIN - 1) + [self.NPC - 128 * (self.NWIN - 1)]
        # A/B view split of the x buffer rows (int16 gather index range)
        self.VIEW = min(32768, self.N)
        self.ABOFS = max(0, self.N - self.VIEW)
        # AllGather chunking: split windows into n_ag_chunks groups
        k = min(n_ag_chunks, self.NWIN)
        bounds = [round(i * self.NWIN / k) for i in range(k + 1)]
        self.WCHUNKS = [(bounds[i], bounds[i + 1]) for i in range(k)]
        self.CROWS = [sum(self.WSIZES[a:b]) for a, b in self.WCHUNKS]
        # padded x row count per window grid (window w occupies rows 128w..)
        # layer table: (name, ic, gathered_cols, relu_out)
        # gathered block is `hid` x-cols (p0: 7 incl na from xc0 buffer)
        self.LAYERS = []
        for name in ['p0', 'p1', 'p2', 'c0', 'c1', 'c2']:
            if name == 'p0':
                ic, g, oc = 7, 7, hid
            elif name == 'c0':
                ic, g, oc = hid + 4, hid, hid
            elif name == 'c2':
                ic, g, oc = hid + 1, hid, out
            else:
                ic, g, oc = hid + 1, hid, hid
            D = ic * hs
            DP = D + (D % 2)  # even pad
            if name == 'p0':
                DP = D + 1 if D % 2 else D
            OCP = oc + (oc % 2)
            self.LAYERS.append(dict(name=name, ic=ic, g=g, oc=oc, D=D, DP=DP,
                                    OCP=OCP, relu=(name != 'c2')))


def _col2orig(cfg, lay):
    """Map plane-major padded column -> original scaling index j=i*HS+h, -1=pad."""
    HS, g, ic, DP = cfg.HS, lay['g'], lay['ic'], lay['DP']
    m = np.full(DP, -1, np.int64)
    if lay['name'] == 'p0':
        for h in range(HS):
            for i in range(g):
                m[h * g + i] = i * HS + h
    elif lay['name'] == 'c0':
        for h in range(HS):
            for i in range(g):
                m[h * g + i] = (3 + i) * HS + h          # fine_x at xc dims 3..
        for f in range(3):
            for h in range(HS):
                m[HS * g + 3 * f + h] = f * HS + h        # fyo
        for h in range(HS):
            m[HS * g + 9 + h] = (ic - 1) * HS + h         # na
    else:
        for h in range(HS):
            for i in range(g):
                m[h * g + i] = i * HS + h
        for h in range(HS):
            m[HS * g + h] = g * HS + h                    # na
    return m


def _balance(items_deg, caps):
    """Greedy: assign items (sorted by degree desc) to bins with capacity,
    minimizing max degree sum. Returns bin index per item."""
    order = np.argsort(-items_deg, kind='stable')
    nbins = len(caps)
    load = np.zeros(nbins)
    cnt = np.zeros(nbins, np.int64)
    out = np.zeros(len(items_deg), np.int64)
    import heapq
    heap = [(0.0, b) for b in range(nbins)]
    heapq.heapify(heap)
    for it in order:
        while True:
            l, b = heapq.heappop(heap)
            if cnt[b] < caps[b]:
                break
        out[it] = b
        cnt[b] += 1
        load[b] += items_deg[it]
        if cnt[b] < caps[b]:
            heapq.heappush(heap, (load[b], b))
    return out


def _preprocess(cfg, inputs):
    N, E, HS = cfg.N, cfg.E, cfg.HS
    ei = np.asarray(inputs['edge_index'])
    src = ei[0].astype(np.int64)
    dst = ei[1].astype(np.int64)
    deg = np.bincount(dst, minlength=N).astype(np.float64)

    node_core = _balance(deg, [cfg.NPC] * NCORES)
    node_win = np.zeros(N, np.int64)
    node_slot = np.zeros(N, np.int64)
    for c in range(NCORES):
        nodes = np.where(node_core == c)[0]
        w = _balance(deg[nodes], cfg.WSIZES)
        node_win[nodes] = w
        for wi in range(cfg.NWIN):
            sel = nodes[w == wi]
            node_slot[sel] = np.arange(len(sel))

    # within-core row and global x row (AG chunk-major, rank-interleaved)
    node_row = node_win * 128 + node_slot
    cbase = np.concatenate([[0], np.cumsum([r * NCORES for r in cfg.CROWS])])
    wchunk = np.zeros(cfg.NWIN, np.int64)
    wofs = np.zeros(cfg.NWIN, np.int64)
    for k, (a, b) in enumerate(cfg.WCHUNKS):
        for w in range(a, b):
            wchunk[w] = k
            wofs[w] = sum(cfg.WSIZES[a:w])
    k_of = wchunk[node_win]
    xrow = (cbase[k_of] + node_core * np.array(cfg.CROWS)[k_of]
            + wofs[node_win] + node_slot)
    xrow_src = xrow[src]

    # edge buckets per (core, window)
    ec = node_core[dst]
    ew = node_win[dst]
    # forced side by xrow of src
    fA = xrow_src < cfg.ABOFS
    fB = xrow_src >= cfg.VIEW

    # per-window global tile structure (max over cores)
    kA = np.zeros(cfg.NWIN, np.int64)
    kB = np.zeros(cfg.NWIN, np.int64)
    tw = np.zeros(cfg.NWIN, np.int64)
    cntA = np.zeros((NCORES, cfg.NWIN), np.int64)
    cntB = np.zeros((NCORES, cfg.NWIN), np.int64)
    cntT = np.zeros((NCORES, cfg.NWIN), np.int64)
    np.add.at(cntA, (ec[fA], ew[fA]), 1)
    np.add.at(cntB, (ec[fB], ew[fB]), 1)
    np.add.at(cntT, (ec, ew), 1)
    for w in range(cfg.NWIN):
        ka = int(np.ceil(cntA[:, w].max() / 128))
        kb = int(np.ceil(cntB[:, w].max() / 128))
        t = max(ka + kb, int(np.ceil(cntT[:, w].max() / 128)), 1)
        kA[w] = ka
        kB[w] = t - ka
        tw[w] = t
        assert kB[w] >= kb
    tbase = np.concatenate([[0], np.cumsum(tw)])
    T = int(tbase[-1])

    ea_np = np.asarray(inputs['edge_attr'], np.float32)
    na_np = np.asarray(inputs['node_attr'], np.float32).reshape(-1)
    fyo_np = np.asarray(inputs['fine_y_orig'], np.float32)

    per_core = []
    for c in range(NCORES):
        ea_s = np.zeros((7, T * 128), np.float32)
        idx_s = np.zeros((16, T * 8), np.int16)
        dst_s = np.full((128, T), 999.0, np.float32)
        na_s = np.zeros((128, T), np.float32)
        fyo_s = np.zeros((128, 3 * T), np.float32)
        for w in range(cfg.NWIN):
            eidx = np.where((ec == c) & (ew == w))[0]
            if len(eidx):
                a_e = eidx[fA[eidx]]
                m_e = eidx[~fA[eidx] & ~fB[eidx]]
                b_e = eidx[fB[eidx]]
                capA = int(kA[w]) * 128
                take = min(len(m_e), capA - len(a_e))
                A = np.concatenate([a_e, m_e[:take]])
                B = np.concatenate([b_e, m_e[take:]])
            else:
                A = B = np.array([], np.int64)
            assert len(A) <= kA[w] * 128 and len(B) <= kB[w] * 128, (w, len(A), len(B))
            for side, edges, ktiles, t0 in (
                    (0, A, int(kA[w]), int(tbase[w])),
                    (1, B, int(kB[w]), int(tbase[w] + kA[w]))):
                nslots = ktiles * 128
                if nslots == 0:
                    continue
                iv = np.zeros(nslots, np.int64)
                iv[:len(edges)] = xrow_src[edges] - (0 if side == 0 else cfg.ABOFS)
                assert iv.min() >= 0 and iv.max() < 32768, (iv.min(), iv.max())
                # slot j (within this side's call) -> tile t0 + j//128, part j%128
                jj = np.arange(nslots)
                tt = t0 + jj // 128
                pp = jj % 128
                idx_s[jj % 16, t0 * 8 + jj // 16] = iv.astype(np.int16)
                if len(edges):
                    e_jj = jj[:len(edges)]
                    e_tt = tt[:len(edges)]
                    e_pp = pp[:len(edges)]
                    ea_s[0:6, e_tt * 128 + e_pp] = ea_np[edges].T
                    ea_s[6, e_tt * 128 + e_pp] = 1.0
                    dst_s[e_pp, e_tt] = node_slot[dst[edges]]
                    na_s[e_pp, e_tt] = na_np[src[edges]]
                    fyo_s[e_pp.repeat(3), (e_tt * 3).repeat(3)
                          + np.tile([0, 1, 2], len(edges))] = fyo_np[src[edges]].ravel()
        per_core.append(dict(ea_s=ea_s, idx_s=np.tile(idx_s, (8, 1)),
                             dst_s=dst_s, na_s=na_s, fyo_s=fyo_s))

    # xc0 buffer: [N, 64] in x-row order: cols [x(5), sdf, na, 0...]
    x_np = np.asarray(inputs['x'], np.float32)
    sdf_np = np.asarray(inputs['sdf'], np.float32)
    xc0 = np.zeros((N, 64), np.float32)
    xc0[xrow, 0:x_np.shape[1]] = x_np
    xc0[xrow, x_np.shape[1]] = sdf_np[:, 0]
    xc0[xrow, x_np.shape[1] + 1] = na_np

    # weights per layer
    wts = {}
    for lay in cfg.LAYERS:
        nm = lay['name']
        win = np.asarray(inputs[f'win_{nm}'], np.float32)
        bin_ = np.asarray(inputs[f'bin_{nm}'], np.float32)
        wout = np.asarray(inputs[f'wout_{nm}'], np.float32)
        bout = np.asarray(inputs[f'bout_{nm}'], np.float32)
        m = _col2orig(cfg, lay)
        DP, OCP = lay['DP'], lay['OCP']
        winT = np.zeros((7, DP), np.float32)
        sel = m >= 0
        winT[0:cfg.EA, sel] = win[m[sel]].T
        winT[6, sel] = bin_[m[sel]]
        woutT = np.zeros((DP + 1, OCP), np.float32)
        woutT[np.where(sel)[0], 0:lay['oc']] = wout[:, m[sel]].T
        woutT[DP, 0:lay['oc']] = bout
        wts[f'winT_{nm}'] = winT
        wts[f'woutT_{nm}'] = woutT

    struct = dict(kA=kA, kB=kB, tw=tw, tbase=tbase, T=T,
                  TWMAX=int(tw.max()))
    asm = dict(node_core=node_core, node_row=node_row)
    return struct, per_core, wts, xc0, asm


def _build(cfg, struct):
    kA, kB, tw, tbase, T = (struct['kA'], struct['kB'], struct['tw'],
                            struct['tbase'], struct['T'])
    TWMAX = struct['TWMAX']
    HID = cfg.HID

    nc = bacc.Bacc("TRN2", target_bir_lowering=False, debug=False,
                   enable_asserts=True, num_devices=NCORES)
    ea_in = nc.dram_tensor("ea_s", [7, T * 128], F32, kind="ExternalInput").ap()
    idx_in = nc.dram_tensor("idx_s", [128, T * 8], I16, kind="ExternalInput").ap()
    dst_in = nc.dram_tensor("dst_s", [128, T], F32, kind="ExternalInput").ap()
    na_in = nc.dram_tensor("na_s", [128, T], F32, kind="ExternalInput").ap()
    fyo_in = nc.dram_tensor("fyo_s", [128, 3 * T], F32, kind="ExternalInput").ap()
    xc0_in = nc.dram_tensor("xc0_in", [cfg.N, 64], F32, kind="ExternalInput").ap()
    win_ins = {}
    wout_ins = {}
    for lay in cfg.LAYERS:
        nm = lay['name']
        win_ins[nm] = nc.dram_tensor(f"winT_{nm}", [7, lay['DP']], F32,
                                     kind="ExternalInput").ap()
        wout_ins[nm] = nc.dram_tensor(f"woutT_{nm}", [lay['DP'] + 1, lay['OCP']],
                                      F32, kind="ExternalInput").ap()
    out_fin = nc.dram_tensor("out_final", [cfg.NPC, cfg.OUT], F32,
                             kind="ExternalOutput").ap()

    DPMAX = max(l['DP'] for l in cfg.LAYERS)
    NFULLMAX = max(l['DP'] // 128 for l in cfg.LAYERS)
    TAILMAX = max(l['DP'] - 128 * (l['DP'] // 128) for l in cfg.LAYERS)

    with tile.TileContext(nc) as tc:
        with (
            tc.tile_pool(name="cst", bufs=1) as cst,
            tc.tile_pool(name="sbw", bufs=1) as sbw,
            tc.tile_pool(name="gst", bufs=2) as gst,
            tc.tile_pool(name="eap", bufs=2) as eap,
            tc.tile_pool(name="scp", bufs=2) as scp,
            tc.tile_pool(name="msgp", bufs=2) as msgp,
            tc.tile_pool(name="Sp", bufs=2) as Sp,
            tc.tile_pool(name="agsp", bufs=2) as agsp,
            tc.tile_pool(name="agtp", bufs=2) as agtp,
            tc.tile_pool(name="outp", bufs=2) as outp,
            tc.tile_pool(name="ps_sc", bufs=2, space="PSUM") as ps_sc,
            tc.tile_pool(name="ps_ag", bufs=1, space="PSUM") as ps_ag,
            tc.tile_pool(name="ps_tp", bufs=1, space="PSUM") as ps_tp,
            tc.tile_pool(name="ps_om", bufs=1, space="PSUM") as ps_om,
            tc.tile_pool(name="dram", bufs=1, space="DRAM") as dram,
        ):
            # ---- constants
            iota_i = cst.tile([128, 128], I32)
            nc.gpsimd.iota(iota_i[:, :], pattern=[[1, 128]], base=0,
                           channel_multiplier=0)
            iota_f = cst.tile([128, 128], F32)
            nc.vector.tensor_copy(iota_f[:, :], iota_i[:, :])
            iota_p = cst.tile([128, 1], I32)
            nc.gpsimd.iota(iota_p[:, :], pattern=[[1, 1]], base=0,
                           channel_multiplier=1)
            iota_pf = cst.tile([128, 1], F32)
            nc.vector.tensor_copy(iota_pf[:, :], iota_p[:, :])
            ident = cst.tile([128, 128], F32R)
            nc.vector.tensor_scalar(out=ident[:, :], in0=iota_f[:, :],
                                    scalar1=iota_pf[:, :], scalar2=None,
                                    op0=ISEQ)
            ones_i = cst.tile([1, 128], I32)
            nc.gpsimd.iota(ones_i[:, :], pattern=[[0, 128]], base=1,
                           channel_multiplier=0)
            ones = cst.tile([1, 128], F32R)
            nc.vector.tensor_copy(ones[:, :], ones_i[:, :])

            # ---- static per-slot data (resident)
            dst_sb = cst.tile([128, T], F32)
            nc.sync.dma_start(out=dst_sb[:, :], in_=dst_in[:, :])
            na_sb = cst.tile([128, T], F32)
            nc.sync.dma_start(out=na_sb[:, :], in_=na_in[:, :])
            fyo_sb = cst.tile([128, 3 * T], F32)
            nc.sync.dma_start(out=fyo_sb[:, :], in_=fyo_in[:, :])
            idx_sb = cst.tile([128, T * 8], I16)
            nc.sync.dma_start(out=idx_sb[:, :], in_=idx_in[:, :])

            # ---- DRAM buffers
            xc0b = dram.tile([cfg.N, 64], F32)
            nc.sync.dma_start(out=xc0b[:, :], in_=xc0_in[:, :])
            X0 = dram.tile([cfg.N, HID], F32)
            X1 = dram.tile([cfg.N, HID], F32)

            gsrc_of = {'p0': (xc0b, 64), 'p1': (X0, HID), 'p2': (X1, HID),
                       'c0': (X0, HID), 'c1': (X1, HID), 'c2': (X0, HID)}
            ag_out_of = {'p0': X0, 'p1': X1, 'p2': X0, 'c0': X1, 'c1': X0}
            cbase = np.concatenate(
                [[0], np.cumsum([r * NCORES for r in cfg.CROWS])]).astype(int)

            for lay in cfg.LAYERS:
                nm, DP, OCP, g = lay['name'], lay['DP'], lay['OCP'], lay['g']
                nfull = DP // 128
                tailk = DP - nfull * 128
                pieces = [(i, min(i + 512, DP)) for i in range(0, DP, 512)]
                gsrc, gcols = gsrc_of[nm]

                # layer weights
                winT_f = sbw.tile([7, DPMAX], F32, tag="winf")
                nc.sync.dma_start(out=winT_f[:, 0:DP], in_=win_ins[nm][:, :])
                winT = sbw.tile([7, DPMAX], F32R, tag="winr")
                nc.scalar.activation(winT[:, 0:DP], winT_f[:, 0:DP], COPY)
                if nfull:
                    wt_f = sbw.tile([128, NFULLMAX * 512], F32, tag="wtf")
                    wt = sbw.tile([128, NFULLMAX * 512], F32R, tag="wtr")
                    for ci in range(nfull):
                        nc.sync.dma_start(
                            out=wt_f[:, ci * OCP:ci * OCP + OCP],
                            in_=wout_ins[nm][ci * 128:(ci + 1) * 128, :])
                    nc.scalar.activation(wt[:, 0:nfull * OCP],
                                         wt_f[:, 0:nfull * OCP], COPY)
                wtail_f = sbw.tile([TAILMAX, 512], F32, tag="wtailf")
                wtail = sbw.tile([TAILMAX, 512], F32R, tag="wtailr")
                if tailk:
                    nc.sync.dma_start(
                        out=wtail_f[0:tailk, 0:OCP],
                        in_=wout_ins[nm][nfull * 128:nfull * 128 + tailk, :])
                    nc.scalar.activation(wtail[0:tailk, 0:OCP],
                                         wtail_f[0:tailk, 0:OCP], COPY)
                wbias_f = sbw.tile([1, 512], F32, tag="wbf")
                nc.sync.dma_start(out=wbias_f[:, 0:OCP],
                                  in_=wout_ins[nm][DP:DP + 1, :])
                wbias = sbw.tile([1, 512], F32R, tag="wbr")
                nc.scalar.activation(wbias[:, 0:OCP], wbias_f[:, 0:OCP], COPY)

                if nm != 'c2':
                    Xout = ag_out_of[nm]
                    oslices = []
                    for k, r in enumerate(cfg.CROWS):
                        t_ = dram.tile([r, HID], F32, tag=f"osl_{nm}_{k}",
                                       name=f"osl_{nm}_{k}")
                        oslices.append(t_)

                for k, (wa, wb) in enumerate(cfg.WCHUNKS):
                    for w in range(wa, wb):
                        nt = int(tw[w])
                        t0 = int(tbase[w])
                        ka, kb = int(kA[w]), int(kB[w])
                        wsz = cfg.WSIZES[w]
                        # gathers (separate narrow staging for p0 so the
                        # gather out AP stays contiguous)
                        xst = gst.tile([128, TWMAX, gcols], F32,
                                       tag=f"xst{gcols}")
                        if ka:
                            nc.gpsimd.dma_gather(
                                out_ap=xst[:, 0:ka, 0:gcols],
                                in_ap=gsrc[0:cfg.VIEW, :],
                                idxs_ap=idx_sb[:, t0 * 8:(t0 + ka) * 8],
                                num_idxs=ka * 128, num_idxs_reg=ka * 128,
                                elem_size=gcols)
                        if kb:
                            nc.gpsimd.dma_gather(
                                out_ap=xst[:, ka:nt, 0:gcols],
                                in_ap=gsrc[cfg.ABOFS:cfg.ABOFS + cfg.VIEW, :],
                                idxs_ap=idx_sb[:, (t0 + ka) * 8:(t0 + nt) * 8],
                                num_idxs=kb * 128, num_idxs_reg=kb * 128,
                                elem_size=gcols)
                        # edge attr
                        eaf = eap.tile([7, TWMAX * 128], F32, tag="eaf")
                        nc.sync.dma_start(out=eaf[:, 0:nt * 128],
                                          in_=ea_in[:, t0 * 128:(t0 + nt) * 128])
                        ear = eap.tile([7, TWMAX * 128], F32R, tag="ear")
                        nc.scalar.activation(ear[:, 0:nt * 128],
                                             eaf[:, 0:nt * 128], COPY)

                        agps = ps_ag.tile([128, DPMAX], F32, tag="agps")
                        for t in range(nt):
                            tg = t0 + t
                            sc = scp.tile([128, DPMAX], F32, tag="sc")
                            for (p0_, p1_) in pieces:
                                scps = ps_sc.tile([128, 512], F32, tag="scps")
                                nc.tensor.matmul(
                                    scps[:, 0:p1_ - p0_],
                                    ear[:, t * 128:(t + 1) * 128],
                                    winT[:, p0_:p1_], start=True, stop=True)
                                nc.scalar.activation(sc[:, p0_:p1_],
                                                     scps[:, 0:p1_ - p0_], RELU)
                            msg = msgp.tile([128, DPMAX], F32R, tag="msg")
                            if nm == 'p0':
                                for h in range(cfg.HS):
                                    lo = h * g
                                    hi = lo + g + (1 if h == cfg.HS - 1 else 0)
                                    nc.vector.tensor_tensor(
                                        out=msg[:, lo:hi], in0=sc[:, lo:hi],
                                        in1=xst[:, t, 0:hi - lo],
                                        op=MULT)
                            else:
                                for h in range(cfg.HS):
                                    nc.vector.tensor_tensor(
                                        out=msg[:, h * g:(h + 1) * g],
                                        in0=sc[:, h * g:(h + 1) * g],
                                        in1=xst[:, t, 0:g], op=MULT)
                                nab = cfg.HS * g + (9 if nm == 'c0' else 0)
                                nc.vector.tensor_tensor(
                                    out=msg[:, nab:DP], in0=sc[:, nab:DP],
                                    in1=na_sb[:, tg:tg + 1].broadcast_to(
                                        [128, DP - nab]), op=MULT)
                                if nm == 'c0':
                                    for f in range(3):
                                        lo = cfg.HS * g + 3 * f
                                        nc.vector.tensor_tensor(
                                            out=msg[:, lo:lo + 3],
                                            in0=sc[:, lo:lo + 3],
                                            in1=fyo_sb[:, 3 * tg + f:3 * tg + f + 1]
                                            .broadcast_to([128, 3]), op=MULT)
                            S = Sp.tile([128, 128], F32R, tag="S")
                            nc.vector.tensor_scalar(
                                out=S[:, :], in0=iota_f[:, :],
                                scalar1=dst_sb[:, tg:tg + 1], scalar2=None,
                                op0=ISEQ)
                            for (p0_, p1_) in pieces:
                                nc.tensor.matmul(
                                    agps[:, p0_:p1_], S[:, :], msg[:, p0_:p1_],
                                    start=(t == 0), stop=(t == nt - 1))
                        # evacuate aggr
                        ags = agsp.tile([128, DPMAX], F32R, tag="ags")
                        nc.scalar.activation(ags[:, 0:DP], agps[:, 0:DP], COPY)
                        # transposes (groups of 4 into one psum tile)
                        agT = agtp.tile([128, NFULLMAX * 128], F32R, tag="agT")
                        agTt = agtp.tile([TAILMAX, 128], F32R, tag="agTt")
                        for gi in range(0, nfull, 4):
                            gn = min(4, nfull - gi)
                            tp = ps_tp.tile([128, 512], F32R, tag="tp")
                            for u in range(gn):
                                ci = gi + u
                                nc.tensor.transpose(
                                    tp[:, u * 128:(u + 1) * 128],
                                    ags[:, ci * 128:(ci + 1) * 128],
                                    ident[:, :])
                            nc.scalar.activation(
                                agT[:, gi * 128:(gi + gn) * 128],
                                tp[:, 0:gn * 128], COPY)
                        if tailk:
                            tp2 = ps_tp.tile([128, 512], F32R, tag="tp")
                            nc.tensor.transpose(
                                tp2[0:tailk, 0:128],
                                ags[:, nfull * 128:nfull * 128 + tailk],
                                ident[:, :])
                            nc.scalar.activation(agTt[0:tailk, :],
                                                 tp2[0:tailk, 0:128], COPY)
                        # out matmul
                        om = ps_om.tile([128, 512], F32, tag="om")
                        for ci in range(nfull):
                            nc.tensor.matmul(
                                om[:, 0:OCP],
                                agT[:, ci * 128:(ci + 1) * 128],
                                wt[:, ci * OCP:(ci + 1) * OCP],
                                start=(ci == 0), stop=False)
                        if tailk:
                            nc.tensor.matmul(om[:, 0:OCP], agTt[0:tailk, :],
                                             wtail[0:tailk, 0:OCP],
                                             start=(nfull == 0), stop=False)
                        nc.tensor.matmul(om[:, 0:OCP], ones[:, :],
                                         wbias[:, 0:OCP], start=False, stop=True)
                        outs = outp.tile([128, 512], F32, tag="outs")
                        if lay['relu']:
                            omr = outp.tile([128, 512], F32, tag="omr")
                            nc.scalar.activation(omr[:, 0:OCP], om[:, 0:OCP],
                                                 RELU)
                            nc.scalar.activation(outs[:, 0:OCP], omr[:, 0:OCP],
                                                 TANH)
                        else:
                            nc.scalar.activation(outs[:, 0:OCP], om[:, 0:OCP],
                                                 TANH)
                        # write out rows
                        if nm == 'c2':
                            nc.sync.dma_start(
                                out=out_fin[w * 128:w * 128 + wsz, :],
                                in_=outs[0:wsz, 0:cfg.OUT])
                        else:
                            ro = sum(cfg.WSIZES[wa:w])
                            nc.sync.dma_start(
                                out=oslices[k][ro:ro + wsz, :],
                                in_=outs[0:wsz, 0:HID])
                    # AllGather for this chunk
                    if nm != 'c2':
                        r = cfg.CROWS[k]
                        nc.gpsimd.collective_compute(
                            "AllGather", mybir.AluOpType.bypass,
                            replica_groups=[list(range(NCORES))],
                            ins=[oslices[k][:, :]],
                            outs=[Xout[cbase[k]:cbase[k] + NCORES * r, :]])
    nc.compile()
    return nc


def _run(inputs, trace=False):
    cfg = Cfg()
    struct, per_core, wts, xc0, asm = _preprocess(cfg, inputs)
    nc = _build(cfg, struct)
    in_maps = []
    for c in range(NCORES):
        im = dict(per_core[c])
        im['xc0_in'] = xc0
        for k, v in wts.items():
            im[k] = v
        in_maps.append(im)
    res = run_bass_kernel_spmd(nc, in_maps, list(range(NCORES)), trace=trace)
    out = np.zeros((cfg.N, cfg.OUT), np.float32)
    for c in range(NCORES):
        sl = res.results[c]['out_final']
        sel = asm['node_core'] == c
        out[sel] = sl[asm['node_row'][sel]]
    return out, res


def kernel(**inputs):
    return _run(inputs, trace=False)[0]
